# revision 11
# baseline (speedup 1.0000x reference)
"""Trainium2 Bass kernel for CrossAttention (B=32, N=M=1024, D=1024, DQK=128).

Computes, per batch b:
    Q = x @ Wq + bq            [N, DQK]
    K = ctx @ Wk + bk          [M, DQK]
    V = ctx @ Wv + bv          [M, D]
    S = Q @ K^T                [N, M]
    W = softmax(S, axis=-1)    [N, M]
    out = W @ V + x            [N, D]
Returns (out, W) as float32, matching the reference.

Sharding: data-parallel over batch across 8 NeuronCores (4 batches/core),
weights replicated. Each core runs an identical SPMD Bass/Tile program.

Schedule: software-pipelined over batches.  While batch b's attention
(scores / softmax / W@V) runs, batch b+1's input DMA, transposes and
Q/K/V projections are interleaved into the same engine queues, so the
PE never idles at phase boundaries.

Precision: Q/K projections and scores run in f32r (fp32 storage, 1
cycle/row on the PE); V projection and W@V run in bf16 with fp32 PSUM
accumulation.  The V bias is folded
into the residual (out = (ub@V0)/sum + (x + bv)), outputs are written to
DRAM as bf16 and widened to fp32 on the host.
"""

import numpy as np

B, N, M, D = 32, 1024, 1024, 1024
E = 128          # DQK
P = 128          # partitions
NCORES = 8
BPC = B // NCORES
KC = D // P      # contraction chunks
NC_ = N // P     # n chunks
MC = M // P      # m chunks
H = 512          # matmul moving free-dim (one PSUM bank of fp32)

_STATE = {}


def _build(nb):
    """Build the per-core Bass/Tile program for nb batches."""
    import concourse.bass as bass
    import concourse.tile as tile
    from concourse import bacc, mybir
    from concourse.masks import make_identity

    f32 = mybir.dt.float32
    f32r = mybir.dt.float32r
    bf16 = mybir.dt.bfloat16
    AX = mybir.AxisListType
    AF = mybir.ActivationFunctionType
    OP = mybir.AluOpType

    nc = bacc.Bacc(None, target_bir_lowering=False, debug=False)
    x_d = nc.dram_tensor("x", [nb, N, D], f32, kind="ExternalInput")
    c_d = nc.dram_tensor("ctx", [nb, M, D], f32, kind="ExternalInput")
    wq_d = nc.dram_tensor("Wq", [D, E], f32, kind="ExternalInput")
    bq_d = nc.dram_tensor("bq", [E], f32, kind="ExternalInput")
    wk_d = nc.dram_tensor("Wk", [D, E], f32, kind="ExternalInput")
    bk_d = nc.dram_tensor("bk", [E], f32, kind="ExternalInput")
    wv_d = nc.dram_tensor("Wv", [D, D], f32, kind="ExternalInput")
    bv_d = nc.dram_tensor("bv", [D], f32, kind="ExternalInput")
    out_d = nc.dram_tensor("out", [nb, N, D], bf16, kind="ExternalOutput")
    wts_d = nc.dram_tensor("wts", [nb, N, M], bf16, kind="ExternalOutput")

    with tile.TileContext(nc) as tc:
        with (
            tc.tile_pool(name="const", bufs=1) as constp,
            tc.tile_pool(name="stage", bufs=3) as stagep,
            tc.tile_pool(name="wstage", bufs=2) as wstagep,
            tc.tile_pool(name="tposed", bufs=1) as tposep,
            tc.tile_pool(name="cbf", bufs=8) as cbfp,
            tc.tile_pool(name="sr", bufs=3) as srp,
            tc.tile_pool(name="vpool", bufs=2) as vpoolp,
            tc.tile_pool(name="xres", bufs=2) as xresp,
            tc.tile_pool(name="qk", bufs=2) as qkp,
            tc.tile_pool(name="attn", bufs=2) as attnp,
            tc.tile_pool(name="outs", bufs=2) as outsp,
            tc.tile_pool(name="small", bufs=16) as smallp,
            tc.tile_pool(name="ps", bufs=3, space="PSUM") as psp,
            tc.tile_pool(name="mm", bufs=2, space="PSUM") as mmp,
            tc.tile_pool(name="pst", bufs=3, space="PSUM") as pstp,
        ):
            # ---- constants ----
            ident_b = constp.tile([P, P], bf16)
            make_identity(nc, ident_b)
            ident_f = constp.tile([P, P], f32)
            make_identity(nc, ident_f)
            ident_r = constp.tile([P, P], f32r)
            nc.vector.tensor_copy(ident_r, ident_f)

            # Small biases via sync queue (cheap), big weights via the
            # scalar HWDGE queue so they stream in parallel with ctx[0].
            bq_sb = constp.tile([P, 1], f32)
            nc.scalar.dma_start(
                out=bq_sb, in_=bq_d[:].rearrange("(p one) -> p one", one=1)
            )
            bk_sb = constp.tile([P, 1], f32)
            nc.scalar.dma_start(
                out=bk_sb, in_=bk_d[:].rearrange("(p one) -> p one", one=1)
            )
            bv_sb = constp.tile([P, D], f32)
            bv_ap = bv_d[:]
            bv_bcast = bass.AP(
                tensor=bv_ap.tensor, offset=bv_ap.offset, ap=[[0, P]] + list(bv_ap.ap)
            )
            nc.gpsimd.dma_start(out=bv_sb, in_=bv_bcast)

            # Wk/Wq as f32r [c-chunk partition, chunk, e]
            wk_sb = constp.tile([P, KC, E], f32r)
            sk = wstagep.tile([P, D], f32, tag="wstage")
            nc.scalar.dma_start(
                out=sk.rearrange("p (k e) -> p k e", k=KC),
                in_=wk_d[:, :].rearrange("(k p) e -> p k e", p=P),
            )
            nc.vector.tensor_copy(wk_sb, sk.rearrange("p (k e) -> p k e", k=KC))
            wq_sb = constp.tile([P, KC, E], f32r)
            sq = wstagep.tile([P, D], f32, tag="wstage")
            nc.scalar.dma_start(
                out=sq.rearrange("p (k e) -> p k e", k=KC),
                in_=wq_d[:, :].rearrange("(k p) e -> p k e", p=P),
            )
            nc.vector.tensor_copy(wq_sb, sq.rearrange("p (k e) -> p k e", k=KC))
            # Wv as bf16 [c-chunk partition, chunk, dout]
            wv_bf = constp.tile([P, KC, D], bf16)
            for k in range(KC):
                s = wstagep.tile([P, D], f32, tag="wstage")
                nc.gpsimd.dma_start(out=s, in_=wv_d[k * P : (k + 1) * P, :])
                nc.scalar.copy(wv_bf[:, k, :], s)

            # per-batch tile handles produced by prep, consumed by attention
            st = [dict() for _ in range(nb)]

            def prep_gen(b, prologue=False):
                """DMA + transpose + K/V/Q projections for batch b.

                Yields between pieces so the driver can interleave this
                work into batch b-1's attention phase.  ctx pieces (sync
                DMA queue) and x pieces (scalar DMA queue) alternate so
                the two input streams transfer in parallel.  In prologue
                mode the V projections are deferred until after all
                transposes so the in-order PE queue is never blocked on
                the Wv weight stream.
                """
                kT = qkp.tile([P, M], f32r, tag="kT")
                qT = qkp.tile([P, N], f32r, tag="qT")
                v_sb = vpoolp.tile([P, MC, D], bf16, tag="v")
                x_res = xresp.tile([P, NC_, D], bf16, tag="xres")
                st[b].update(kT=kT, qT=qT, v=v_sb, xres=x_res)
                cbfs = [None] * MC

                def v_piece(j):
                    # V projection for m-block j (contract over all of D)
                    for h in range(2):
                        vps = mmp.tile([P, H], f32, tag="mm")
                        for k in range(KC):
                            nc.tensor.matmul(
                                vps,
                                cbfs[j][:, k, :],
                                wv_bf[:, k, h * H : (h + 1) * H],
                                start=(k == 0),
                                stop=(k == KC - 1),
                            )
                        nc.scalar.copy(v_sb[:, j, h * H : (h + 1) * H], vps)

                def ctx_pieces():
                    ctxTh = None
                    for j in range(MC):
                        s = stagep.tile([P, D], f32, tag="stage")
                        eng = nc.scalar if (prologue and j % 2) else nc.sync
                        eng.dma_start(out=s, in_=c_d[b, j * P : (j + 1) * P, :])
                        # round to f32r once so the transposes stream at
                        # 1.5 cycles/row instead of 2.0 for strict f32
                        sr = srp.tile([P, D], f32r, tag="sr")
                        nc.scalar.copy(sr, s)
                        jj = j % 4
                        if jj == 0:
                            ctxTh = tposep.tile([P, KC, H], f32r, tag="ctxT")
                        cbf = cbfp.tile([P, KC, P], bf16, tag="cbf")
                        cbfs[j] = cbf
                        for g in range(2):
                            pt = pstp.tile([P, 4, P], f32r, tag="t")
                            for u in range(4):
                                i = 4 * g + u
                                nc.tensor.transpose(
                                    pt[:, u, :], sr[:, i * P : (i + 1) * P], ident_r
                                )
                            nc.vector.tensor_copy(
                                ctxTh[:, 4 * g : 4 * g + 4, jj * P : (jj + 1) * P], pt
                            )
                            nc.scalar.copy(cbf[:, 4 * g : 4 * g + 4, :], pt)
                        yield
                        if not prologue:
                            v_piece(j)
                            yield
                        if jj == 3:
                            hh = j // 4
                            kps = mmp.tile([P, H], f32, tag="mm")
                            for k in range(KC):
                                nc.tensor.matmul(
                                    kps,
                                    wk_sb[:, k, :],
                                    ctxTh[:, k, :],
                                    start=(k == 0),
                                    stop=(k == KC - 1),
                                )
                            nc.scalar.add(kT[:, hh * H : (hh + 1) * H], kps, bk_sb)
                            yield

                def x_pieces():
                    xTh = None
                    for j in range(NC_):
                        s = stagep.tile([P, D], f32, tag="stage")
                        nc.scalar.dma_start(out=s, in_=x_d[b, j * P : (j + 1) * P, :])
                        # residual with the V bias folded in: x + bv (bf16)
                        nc.gpsimd.tensor_tensor(x_res[:, j, :], s, bv_sb, op=OP.add)
                        sr = srp.tile([P, D], f32r, tag="sr")
                        nc.scalar.copy(sr, s)
                        jj = j % 4
                        if jj == 0:
                            xTh = tposep.tile([P, KC, H], f32r, tag="xT")
                        for g in range(2):
                            pt = pstp.tile([P, 4, P], f32r, tag="t")
                            for u in range(4):
                                i = 4 * g + u
                                nc.tensor.transpose(
                                    pt[:, u, :], sr[:, i * P : (i + 1) * P], ident_r
                                )
                            nc.vector.tensor_copy(
                                xTh[:, 4 * g : 4 * g + 4, jj * P : (jj + 1) * P], pt
                            )
                        yield
                        if jj == 3:
                            hh = j // 4
                            qps = mmp.tile([P, H], f32, tag="mm")
                            for k in range(KC):
                                nc.tensor.matmul(
                                    qps,
                                    wq_sb[:, k, :],
                                    xTh[:, k, :],
                                    start=(k == 0),
                                    stop=(k == KC - 1),
                                )
                            nc.scalar.add(qT[:, hh * H : (hh + 1) * H], qps, bq_sb)
                            yield

                cg, xg = ctx_pieces(), x_pieces()
                done = object()
                while True:
                    a = next(cg, done)
                    if a is not done:
                        yield
                    bb = next(xg, done)
                    if bb is not done:
                        yield
                    if a is done and bb is done:
                        break
                if prologue:
                    for j in range(MC):
                        v_piece(j)
                        yield

            def attn_gen(b):
                kT, qT, v_sb, x_res = (st[b][k] for k in ("kT", "qT", "v", "xres"))

                def emit_scores(i):
                    s0 = psp.tile([P, H], f32, tag="ps")
                    s1 = psp.tile([P, H], f32, tag="ps")
                    nc.tensor.matmul(s0, qT[:, i * P : (i + 1) * P], kT[:, 0:H])
                    nc.tensor.matmul(s1, qT[:, i * P : (i + 1) * P], kT[:, H:M])
                    return s0, s1

                s_tiles = emit_scores(0)
                for i in range(NC_):
                    s0, s1 = s_tiles
                    # softmax without max-subtraction: scores are O(25) so
                    # exp stays well inside fp32/bf16 range.  ub holds the
                    # unnormalized exp (bf16); 1/sum folds into the outputs.
                    ub = attnp.tile([P, M], bf16, tag="ub")
                    sx0 = smallp.tile([P, 1], f32, tag="sm")
                    sx1 = smallp.tile([P, 1], f32, tag="sm")
                    nc.scalar.activation(
                        ub[:, 0:H], s0, AF.Exp, bias=0.0, scale=1.0, accum_out=sx0
                    )
                    nc.scalar.activation(
                        ub[:, H:M], s1, AF.Exp, bias=0.0, scale=1.0, accum_out=sx1
                    )
                    sumex = smallp.tile([P, 1], f32, tag="sm")
                    nc.vector.tensor_add(sumex, sx0, sx1)
                    rsum = smallp.tile([P, 1], f32, tag="sm")
                    nc.vector.reciprocal(rsum, sumex)
                    yield
                    # next chunk's scores go on the PE queue *before* this
                    # chunk's weight-transpose, so softmax(i+1) overlaps the
                    # W@V matmuls below.
                    if i + 1 < NC_:
                        s_tiles = emit_scores(i + 1)
                    pstW = pstp.tile([P, MC, P], bf16, tag="t")
                    for j in range(MC):
                        nc.tensor.transpose(
                            pstW[:, j, :], ub[:, j * P : (j + 1) * P], ident_b
                        )
                    pT = attnp.tile([P, MC, P], bf16, tag="pT")
                    nc.vector.tensor_copy(pT, pstW)
                    att = outsp.tile([P, D], bf16, tag="att")
                    for h in range(2):
                        av = mmp.tile([P, H], f32, tag="mm")
                        for j in range(MC):
                            nc.tensor.matmul(
                                av,
                                pT[:, j, :],
                                v_sb[:, j, h * H : (h + 1) * H],
                                start=(j == 0),
                                stop=(j == MC - 1),
                            )
                        nc.vector.scalar_tensor_tensor(
                            att[:, h * H : (h + 1) * H],
                            av,
                            rsum,
                            x_res[:, i, h * H : (h + 1) * H],
                            op0=OP.mult,
                            op1=OP.add,
                        )
                    # normalized weights -> DRAM (bf16), off the critical path;
                    # writes go via the GpSimd software DGE so they never block
                    # the sync queue's input reads.
                    pw = outsp.tile([P, M], bf16, tag="pw")
                    nc.vector.tensor_scalar_mul(pw, ub, rsum)
                    nc.gpsimd.dma_start(out=wts_d[b, i * P : (i + 1) * P, :], in_=pw)
                    nc.sync.dma_start(out=out_d[b, i * P : (i + 1) * P, :], in_=att)
                    yield

            # ---- prologue: batch 0 prep, serial ----
            for _ in prep_gen(0, prologue=True):
                pass
            # ---- pipelined main loop ----
            _DONE = object()
            for b in range(nb):
                pg = prep_gen(b + 1) if b + 1 < nb else None
                for _ in attn_gen(b):
                    if pg is not None:
                        for _ in range(2):
                            if next(pg, _DONE) is _DONE:
                                pg = None
                                break
                if pg is not None:
                    for _ in pg:
                        pass

    return nc


def _get_program(nb):
    if nb not in _STATE:
        nc = _build(nb)
        nc.finalize()
        _STATE[nb] = nc
    return _STATE[nb]


def run(inputs, trace=False):
    """Run on 8 cores; returns (out, wts, BassKernelResults)."""
    from concourse import bass_utils

    nc = _get_program(BPC)
    x = np.ascontiguousarray(np.asarray(inputs["x"], dtype=np.float32))
    ctx = np.ascontiguousarray(np.asarray(inputs["context"], dtype=np.float32))
    shared = {
        "Wq": np.ascontiguousarray(np.asarray(inputs["Wq"], dtype=np.float32)),
        "bq": np.ascontiguousarray(np.asarray(inputs["bq"], dtype=np.float32)),
        "Wk": np.ascontiguousarray(np.asarray(inputs["Wk"], dtype=np.float32)),
        "bk": np.ascontiguousarray(np.asarray(inputs["bk"], dtype=np.float32)),
        "Wv": np.ascontiguousarray(np.asarray(inputs["Wv"], dtype=np.float32)),
        "bv": np.ascontiguousarray(np.asarray(inputs["bv"], dtype=np.float32)),
    }
    in_maps = []
    for c in range(NCORES):
        m = dict(shared)
        m["x"] = x[c * BPC : (c + 1) * BPC]
        m["ctx"] = ctx[c * BPC : (c + 1) * BPC]
        in_maps.append(m)

    kw = {}
    if trace:
        _install_ntff_hook()
        kw["trace"] = True
    res = bass_utils.run_bass_kernel_spmd(nc, in_maps, list(range(NCORES)), **kw)
    out = np.concatenate(
        [np.asarray(res.results[c]["out"], dtype=np.float32) for c in range(NCORES)],
        axis=0,
    )
    wts = np.concatenate(
        [np.asarray(res.results[c]["wts"], dtype=np.float32) for c in range(NCORES)],
        axis=0,
    )
    return out, wts, res


def _install_ntff_hook():
    """The container's antenv stub lacks axon_hooks; provide it so
    run_bass_kernel_spmd(trace=True) can capture NTFF profiles."""
    import sys, types

    if "antenv.axon_hooks" in sys.modules:
        return
    import antenv
    from concourse import bass_utils

    bass_utils.upload_artifacts = lambda d: d  # no artifact store here
    try:
        from trn_agent_boot.trn_boot import _ntff_profile_via_ctypes

        hook = _ntff_profile_via_ctypes("/opt/axon/libaxon_pjrt.so")
    except Exception:
        hook = None
    mod = types.ModuleType("antenv.axon_hooks")
    mod.get_axon_ntff_profile_hook = lambda: hook
    mod.set_axon_ntff_profile_hook = lambda h: None
    sys.modules["antenv.axon_hooks"] = mod
    antenv.axon_hooks = mod


def kernel(**inputs):
    out, wts, _ = run(inputs, trace=False)
    return out, wts


# revision 12
# speedup vs baseline: 1.0311x; 1.0311x over previous
"""Trainium2 Bass kernel for CrossAttention (B=32, N=M=1024, D=1024, DQK=128).

Computes, per batch b:
    Q = x @ Wq + bq            [N, DQK]
    K = ctx @ Wk + bk          [M, DQK]
    V = ctx @ Wv + bv          [M, D]
    S = Q @ K^T                [N, M]
    W = softmax(S, axis=-1)    [N, M]
    out = W @ V + x            [N, D]
Returns (out, W) as float32, matching the reference.

Sharding: data-parallel over batch across 8 NeuronCores (4 batches/core),
weights replicated. Each core runs an identical SPMD Bass/Tile program.

Schedule: software-pipelined over batches.  While batch b's attention
(scores / softmax / W@V) runs, batch b+1's input DMA, transposes and
Q/K/V projections are interleaved into the same engine queues, so the
PE never idles at phase boundaries.

Precision: Q/K projections and scores run in f32r (fp32 storage, 1
cycle/row on the PE); V projection and W@V run in bf16 with fp32 PSUM
accumulation.  The V bias is folded
into the residual (out = (ub@V0)/sum + (x + bv)), outputs are written to
DRAM as bf16 and widened to fp32 on the host.
"""

import numpy as np

B, N, M, D = 32, 1024, 1024, 1024
E = 128          # DQK
P = 128          # partitions
NCORES = 8
BPC = B // NCORES
KC = D // P      # contraction chunks
NC_ = N // P     # n chunks
MC = M // P      # m chunks
H = 512          # matmul moving free-dim (one PSUM bank of fp32)

_STATE = {}


def _build(nb):
    """Build the per-core Bass/Tile program for nb batches."""
    import concourse.bass as bass
    import concourse.tile as tile
    from concourse import bacc, mybir
    from concourse.masks import make_identity

    f32 = mybir.dt.float32
    f32r = mybir.dt.float32r
    bf16 = mybir.dt.bfloat16
    AX = mybir.AxisListType
    AF = mybir.ActivationFunctionType
    OP = mybir.AluOpType

    nc = bacc.Bacc(None, target_bir_lowering=False, debug=False)
    x_d = nc.dram_tensor("x", [nb, N, D], f32, kind="ExternalInput")
    c_d = nc.dram_tensor("ctx", [nb, M, D], f32, kind="ExternalInput")
    wq_d = nc.dram_tensor("Wq", [D, E], f32, kind="ExternalInput")
    bq_d = nc.dram_tensor("bq", [E], f32, kind="ExternalInput")
    wk_d = nc.dram_tensor("Wk", [D, E], f32, kind="ExternalInput")
    bk_d = nc.dram_tensor("bk", [E], f32, kind="ExternalInput")
    wv_d = nc.dram_tensor("Wv", [D, D], f32, kind="ExternalInput")
    bv_d = nc.dram_tensor("bv", [D], f32, kind="ExternalInput")
    out_d = nc.dram_tensor("out", [nb, N, D], bf16, kind="ExternalOutput")
    wts_d = nc.dram_tensor("wts", [nb, N, M], bf16, kind="ExternalOutput")

    with tile.TileContext(nc) as tc:
        with (
            tc.tile_pool(name="const", bufs=1) as constp,
            tc.tile_pool(name="stage", bufs=3) as stagep,
            tc.tile_pool(name="wstage", bufs=2) as wstagep,
            tc.tile_pool(name="tposed", bufs=1) as tposep,
            tc.tile_pool(name="cbf", bufs=8) as cbfp,
            tc.tile_pool(name="vpool", bufs=2) as vpoolp,
            tc.tile_pool(name="xres", bufs=2) as xresp,
            tc.tile_pool(name="qk", bufs=2) as qkp,
            tc.tile_pool(name="attn", bufs=2) as attnp,
            tc.tile_pool(name="outs", bufs=2) as outsp,
            tc.tile_pool(name="small", bufs=16) as smallp,
            tc.tile_pool(name="ps", bufs=3, space="PSUM") as psp,
            tc.tile_pool(name="mm", bufs=2, space="PSUM") as mmp,
            tc.tile_pool(name="pst", bufs=3, space="PSUM") as pstp,
        ):
            # ---- constants ----
            ident_b = constp.tile([P, P], bf16)
            make_identity(nc, ident_b)
            ident_f = constp.tile([P, P], f32)
            make_identity(nc, ident_f)

            # Small biases via sync queue (cheap), big weights via the
            # scalar HWDGE queue so they stream in parallel with ctx[0].
            bq_sb = constp.tile([P, 1], f32)
            nc.scalar.dma_start(
                out=bq_sb, in_=bq_d[:].rearrange("(p one) -> p one", one=1)
            )
            bk_sb = constp.tile([P, 1], f32)
            nc.scalar.dma_start(
                out=bk_sb, in_=bk_d[:].rearrange("(p one) -> p one", one=1)
            )
            bv_sb = constp.tile([P, D], f32)
            bv_ap = bv_d[:]
            bv_bcast = bass.AP(
                tensor=bv_ap.tensor, offset=bv_ap.offset, ap=[[0, P]] + list(bv_ap.ap)
            )
            nc.gpsimd.dma_start(out=bv_sb, in_=bv_bcast)

            # Wk/Wq as f32r [c-chunk partition, chunk, e]
            wk_sb = constp.tile([P, KC, E], f32r)
            sk = wstagep.tile([P, D], f32, tag="wstage")
            nc.scalar.dma_start(
                out=sk.rearrange("p (k e) -> p k e", k=KC),
                in_=wk_d[:, :].rearrange("(k p) e -> p k e", p=P),
            )
            nc.vector.tensor_copy(wk_sb, sk.rearrange("p (k e) -> p k e", k=KC))
            wq_sb = constp.tile([P, KC, E], f32r)
            sq = wstagep.tile([P, D], f32, tag="wstage")
            nc.scalar.dma_start(
                out=sq.rearrange("p (k e) -> p k e", k=KC),
                in_=wq_d[:, :].rearrange("(k p) e -> p k e", p=P),
            )
            nc.vector.tensor_copy(wq_sb, sq.rearrange("p (k e) -> p k e", k=KC))
            # Wv as bf16 [c-chunk partition, chunk, dout]
            wv_bf = constp.tile([P, KC, D], bf16)
            for k in range(KC):
                s = wstagep.tile([P, D], f32, tag="wstage")
                nc.gpsimd.dma_start(out=s, in_=wv_d[k * P : (k + 1) * P, :])
                nc.scalar.copy(wv_bf[:, k, :], s)

            # per-batch tile handles produced by prep, consumed by attention
            st = [dict() for _ in range(nb)]

            def prep_gen(b, prologue=False):
                """DMA + transpose + K/V/Q projections for batch b.

                Yields between pieces so the driver can interleave this
                work into batch b-1's attention phase.  ctx pieces (sync
                DMA queue) and x pieces (scalar DMA queue) alternate so
                the two input streams transfer in parallel.  In prologue
                mode the V projections are deferred until after all
                transposes so the in-order PE queue is never blocked on
                the Wv weight stream.
                """
                kT = qkp.tile([P, M], f32r, tag="kT")
                qT = qkp.tile([P, N], f32r, tag="qT")
                v_sb = vpoolp.tile([P, MC, D], bf16, tag="v")
                x_res = xresp.tile([P, NC_, D], bf16, tag="xres")
                st[b].update(kT=kT, qT=qT, v=v_sb, xres=x_res)
                cbfs = [None] * MC

                def v_piece(j):
                    # V projection for m-block j (contract over all of D)
                    for h in range(2):
                        vps = mmp.tile([P, H], f32, tag="mm")
                        for k in range(KC):
                            nc.tensor.matmul(
                                vps,
                                cbfs[j][:, k, :],
                                wv_bf[:, k, h * H : (h + 1) * H],
                                start=(k == 0),
                                stop=(k == KC - 1),
                            )
                        nc.scalar.copy(v_sb[:, j, h * H : (h + 1) * H], vps)

                def ctx_pieces():
                    ctxTh = None
                    for j in range(MC):
                        s = stagep.tile([P, D], f32, tag="stage")
                        eng = nc.scalar if (prologue and j % 2) else nc.sync
                        eng.dma_start(out=s, in_=c_d[b, j * P : (j + 1) * P, :])
                        jj = j % 4
                        if jj == 0:
                            ctxTh = tposep.tile([P, KC, H], f32r, tag="ctxT")
                        cbf = cbfp.tile([P, KC, P], bf16, tag="cbf")
                        cbfs[j] = cbf
                        for g in range(2):
                            pt = pstp.tile([P, 4, P], f32, tag="t")
                            for u in range(4):
                                i = 4 * g + u
                                nc.tensor.transpose(
                                    pt[:, u, :], s[:, i * P : (i + 1) * P], ident_f
                                )
                            nc.vector.tensor_copy(
                                ctxTh[:, 4 * g : 4 * g + 4, jj * P : (jj + 1) * P], pt
                            )
                            nc.scalar.copy(cbf[:, 4 * g : 4 * g + 4, :], pt)
                        yield
                        if not prologue:
                            v_piece(j)
                            yield
                        if jj == 3:
                            hh = j // 4
                            kps = mmp.tile([P, H], f32, tag="mm")
                            for k in range(KC):
                                nc.tensor.matmul(
                                    kps,
                                    wk_sb[:, k, :],
                                    ctxTh[:, k, :],
                                    start=(k == 0),
                                    stop=(k == KC - 1),
                                )
                            nc.scalar.add(kT[:, hh * H : (hh + 1) * H], kps, bk_sb)
                            yield

                def x_pieces():
                    xTh = None
                    for j in range(NC_):
                        s = stagep.tile([P, D], f32, tag="stage")
                        nc.scalar.dma_start(out=s, in_=x_d[b, j * P : (j + 1) * P, :])
                        # residual with the V bias folded in: x + bv (bf16)
                        nc.gpsimd.tensor_tensor(x_res[:, j, :], s, bv_sb, op=OP.add)
                        jj = j % 4
                        if jj == 0:
                            xTh = tposep.tile([P, KC, H], f32r, tag="xT")
                        for g in range(2):
                            pt = pstp.tile([P, 4, P], f32, tag="t")
                            for u in range(4):
                                i = 4 * g + u
                                nc.tensor.transpose(
                                    pt[:, u, :], s[:, i * P : (i + 1) * P], ident_f
                                )
                            nc.vector.tensor_copy(
                                xTh[:, 4 * g : 4 * g + 4, jj * P : (jj + 1) * P], pt
                            )
                        yield
                        if jj == 3:
                            hh = j // 4
                            qps = mmp.tile([P, H], f32, tag="mm")
                            for k in range(KC):
                                nc.tensor.matmul(
                                    qps,
                                    wq_sb[:, k, :],
                                    xTh[:, k, :],
                                    start=(k == 0),
                                    stop=(k == KC - 1),
                                )
                            nc.scalar.add(qT[:, hh * H : (hh + 1) * H], qps, bq_sb)
                            yield

                cg, xg = ctx_pieces(), x_pieces()
                done = object()
                while True:
                    a = next(cg, done)
                    if a is not done:
                        yield
                    bb = next(xg, done)
                    if bb is not done:
                        yield
                    if a is done and bb is done:
                        break
                if prologue:
                    for j in range(MC):
                        v_piece(j)
                        yield

            def attn_gen(b):
                kT, qT, v_sb, x_res = (st[b][k] for k in ("kT", "qT", "v", "xres"))

                def emit_scores(i):
                    s0 = psp.tile([P, H], f32, tag="ps")
                    s1 = psp.tile([P, H], f32, tag="ps")
                    nc.tensor.matmul(s0, qT[:, i * P : (i + 1) * P], kT[:, 0:H])
                    nc.tensor.matmul(s1, qT[:, i * P : (i + 1) * P], kT[:, H:M])
                    return s0, s1

                s_tiles = emit_scores(0)
                for i in range(NC_):
                    s0, s1 = s_tiles
                    # softmax without max-subtraction: scores are O(25) so
                    # exp stays well inside fp32/bf16 range.  ub holds the
                    # unnormalized exp (bf16); 1/sum folds into the outputs.
                    ub = attnp.tile([P, M], bf16, tag="ub")
                    sx0 = smallp.tile([P, 1], f32, tag="sm")
                    sx1 = smallp.tile([P, 1], f32, tag="sm")
                    nc.scalar.activation(
                        ub[:, 0:H], s0, AF.Exp, bias=0.0, scale=1.0, accum_out=sx0
                    )
                    nc.scalar.activation(
                        ub[:, H:M], s1, AF.Exp, bias=0.0, scale=1.0, accum_out=sx1
                    )
                    sumex = smallp.tile([P, 1], f32, tag="sm")
                    nc.vector.tensor_add(sumex, sx0, sx1)
                    rsum = smallp.tile([P, 1], f32, tag="sm")
                    nc.vector.reciprocal(rsum, sumex)
                    yield
                    # next chunk's scores go on the PE queue *before* this
                    # chunk's weight-transpose, so softmax(i+1) overlaps the
                    # W@V matmuls below.
                    if i + 1 < NC_:
                        s_tiles = emit_scores(i + 1)
                    pstW = pstp.tile([P, MC, P], bf16, tag="t")
                    for j in range(MC):
                        nc.tensor.transpose(
                            pstW[:, j, :], ub[:, j * P : (j + 1) * P], ident_b
                        )
                    pT = attnp.tile([P, MC, P], bf16, tag="pT")
                    nc.vector.tensor_copy(pT, pstW)
                    att = outsp.tile([P, D], bf16, tag="att")
                    for h in range(2):
                        av = mmp.tile([P, H], f32, tag="mm")
                        for j in range(MC):
                            nc.tensor.matmul(
                                av,
                                pT[:, j, :],
                                v_sb[:, j, h * H : (h + 1) * H],
                                start=(j == 0),
                                stop=(j == MC - 1),
                            )
                        nc.vector.scalar_tensor_tensor(
                            att[:, h * H : (h + 1) * H],
                            av,
                            rsum,
                            x_res[:, i, h * H : (h + 1) * H],
                            op0=OP.mult,
                            op1=OP.add,
                        )
                    # normalized weights -> DRAM (bf16), off the critical path;
                    # writes go via the GpSimd software DGE so they never block
                    # the sync queue's input reads.
                    pw = outsp.tile([P, M], bf16, tag="pw")
                    nc.vector.tensor_scalar_mul(pw, ub, rsum)
                    nc.gpsimd.dma_start(out=wts_d[b, i * P : (i + 1) * P, :], in_=pw)
                    nc.sync.dma_start(out=out_d[b, i * P : (i + 1) * P, :], in_=att)
                    yield

            # ---- prologue: batch 0 prep, serial ----
            for _ in prep_gen(0, prologue=True):
                pass
            # ---- pipelined main loop ----
            _DONE = object()
            for b in range(nb):
                pg = prep_gen(b + 1) if b + 1 < nb else None
                for _ in attn_gen(b):
                    if pg is not None:
                        for _ in range(2):
                            if next(pg, _DONE) is _DONE:
                                pg = None
                                break
                if pg is not None:
                    for _ in pg:
                        pass

    return nc


def _get_program(nb):
    if nb not in _STATE:
        nc = _build(nb)
        nc.finalize()
        _STATE[nb] = nc
    return _STATE[nb]


def run(inputs, trace=False):
    """Run on 8 cores; returns (out, wts, BassKernelResults)."""
    from concourse import bass_utils

    nc = _get_program(BPC)
    x = np.ascontiguousarray(np.asarray(inputs["x"], dtype=np.float32))
    ctx = np.ascontiguousarray(np.asarray(inputs["context"], dtype=np.float32))
    shared = {
        "Wq": np.ascontiguousarray(np.asarray(inputs["Wq"], dtype=np.float32)),
        "bq": np.ascontiguousarray(np.asarray(inputs["bq"], dtype=np.float32)),
        "Wk": np.ascontiguousarray(np.asarray(inputs["Wk"], dtype=np.float32)),
        "bk": np.ascontiguousarray(np.asarray(inputs["bk"], dtype=np.float32)),
        "Wv": np.ascontiguousarray(np.asarray(inputs["Wv"], dtype=np.float32)),
        "bv": np.ascontiguousarray(np.asarray(inputs["bv"], dtype=np.float32)),
    }
    in_maps = []
    for c in range(NCORES):
        m = dict(shared)
        m["x"] = x[c * BPC : (c + 1) * BPC]
        m["ctx"] = ctx[c * BPC : (c + 1) * BPC]
        in_maps.append(m)

    kw = {}
    if trace:
        _install_ntff_hook()
        kw["trace"] = True
    res = bass_utils.run_bass_kernel_spmd(nc, in_maps, list(range(NCORES)), **kw)
    out = np.concatenate(
        [np.asarray(res.results[c]["out"], dtype=np.float32) for c in range(NCORES)],
        axis=0,
    )
    wts = np.concatenate(
        [np.asarray(res.results[c]["wts"], dtype=np.float32) for c in range(NCORES)],
        axis=0,
    )
    return out, wts, res


def _install_ntff_hook():
    """The container's antenv stub lacks axon_hooks; provide it so
    run_bass_kernel_spmd(trace=True) can capture NTFF profiles."""
    import sys, types

    if "antenv.axon_hooks" in sys.modules:
        return
    import antenv
    from concourse import bass_utils

    bass_utils.upload_artifacts = lambda d: d  # no artifact store here
    try:
        from trn_agent_boot.trn_boot import _ntff_profile_via_ctypes

        hook = _ntff_profile_via_ctypes("/opt/axon/libaxon_pjrt.so")
    except Exception:
        hook = None
    mod = types.ModuleType("antenv.axon_hooks")
    mod.get_axon_ntff_profile_hook = lambda: hook
    mod.set_axon_ntff_profile_hook = lambda h: None
    sys.modules["antenv.axon_hooks"] = mod
    antenv.axon_hooks = mod


def kernel(**inputs):
    out, wts, _ = run(inputs, trace=False)
    return out, wts
